# revision 1
# baseline (speedup 1.0000x reference)
"""BERT-LSTM-CRF kernel for Trainium2, 8 NeuronCores.

Sharding: direction x batch split. Cores 0-3: forward LSTM over batch
quarters (16 samples each); cores 4-7: backward LSTM over the same
quarters (fed time-reversed embeddings so the same SPMD program runs
everywhere). Each core: input GEMM (embeds @ W_ih^T + bias via an
augmented ones-column), 512-step LSTM scan (PE recurrent matmul in
fp32r, moving operand = W_hh^T streamed at 1 cyc/row), and the output
projection of its half of the hidden concat. Host does the (pure data
movement) wordpiece gather / alignment, time reversal for the backward
cores, and the final fwd+bwd partial-sum + bias add.
"""
import os
import sys
import numpy as np

sys.path.insert(0, "/opt/trn_rl_repo")

B, S, D, H, T = 64, 512, 768, 384, 22
G4 = 4 * H            # 1536 gate rows
DA = 896              # 768 + 1 ones-column + zero pad to 7*128
BL = 16               # batch per core
NC = 8
STEPS = int(os.environ.get("KSTEPS", str(S)))
MM_DTYPE = os.environ.get("KMMDT", "float32r")  # float32r | float32
XG_CHUNK = 2          # scan timesteps per xg DMA chunk

_cache = {}


def _align_np(hidden_states, start_ids, masks):
    """numpy port of reference._align."""
    hs = np.asarray(hidden_states)
    sid = np.asarray(start_ids)
    msk = np.asarray(masks)
    Bb, Ss, _ = hs.shape
    t = np.arange(Ss)[None, :]
    valid = sid >= 0
    n = valid.sum(-1)
    last_sid = np.take_along_axis(sid, (n - 1)[:, None], axis=1)
    idx = np.where(t == 0, 0,
          np.where(t < n[:, None], sid - 1,
          np.where(t == n[:, None], last_sid, 0)))
    idx = np.clip(idx, 0, Ss - 1).astype(np.int64)
    gathered = np.take_along_axis(hs, idx[:, :, None], axis=1)
    sent_len = msk.sum(-1)
    keep = (t < sent_len[:, None])[:, :, None]
    return np.where(keep, gathered, 0.0).astype(np.float32)


def _build_program():
    from concourse import bass, bacc, tile, mybir
    from contextlib import ExitStack

    f32 = mybir.dt.float32
    mmdt = getattr(mybir.dt, MM_DTYPE)
    AF = mybir.ActivationFunctionType

    nc = bacc.Bacc("TRN2", target_bir_lowering=False, debug=False,
                   num_devices=NC)

    emb = nc.dram_tensor("emb", [BL * S, DA], f32, kind="ExternalInput")
    wih = nc.dram_tensor("wih", [DA, G4], mmdt, kind="ExternalInput")
    whh = nc.dram_tensor("whh", [H, G4], mmdt, kind="ExternalInput")
    wlin = nc.dram_tensor("wlin", [H, T], mmdt, kind="ExternalInput")
    h0t = nc.dram_tensor("h0t", [H, BL], mmdt, kind="ExternalInput")
    c0 = nc.dram_tensor("c0", [BL, H], f32, kind="ExternalInput")
    id16 = nc.dram_tensor("id16", [16, 16], f32, kind="ExternalInput")
    id128 = nc.dram_tensor("id128", [128, 128], f32, kind="ExternalInput")
    partial = nc.dram_tensor("partial", [S * BL, T], f32,
                             kind="ExternalOutput")
    xg_dram = nc.dram_tensor("xg_scratch", [S, BL, G4], f32)

    RT = (BL * S) // 128  # 64 row tiles of the input GEMM
    KD = DA // 128        # 7 contraction chunks (incl. bias/pad)
    KH = H // 128         # 3 hidden chunks

    with tile.TileContext(nc) as tc, ExitStack() as big:
        # --- persistent SBUF tiles ---
        consts = big.enter_context(tc.tile_pool(name="consts", bufs=1))
        hist_pool = big.enter_context(tc.tile_pool(name="hist", bufs=1))

        id16_sb = consts.tile([16, 16], f32, tag="id16")
        nc.sync.dma_start(id16_sb[:], id16[:])
        id128_sb = consts.tile([128, 128], f32, tag="id128")
        nc.sync.dma_start(id128_sb[:], id128[:])
        whh_all = consts.tile([128, KH, G4], mmdt, tag="whh")
        nc.sync.dma_start(whh_all[:],
                          whh.rearrange("(k p) g -> p k g", p=128))
        whh_sb = [whh_all[:, k, :] for k in range(KH)]
        h0t_all = consts.tile([128, KH, BL], mmdt, tag="h0t")
        nc.sync.dma_start(h0t_all[:],
                          h0t.rearrange("(k p) b -> p k b", p=128))
        h0t_sb = [h0t_all[:, k, :] for k in range(KH)]
        c0_sb = consts.tile([BL, H], f32, tag="c0")
        nc.sync.dma_start(c0_sb[:], c0[:])
        wlin_all = consts.tile([128, KH, T], mmdt, tag="wlin")
        nc.sync.dma_start(wlin_all[:],
                          wlin.rearrange("(k p) t -> p k t", p=128))
        wlin_sb = [wlin_all[:, k, :] for k in range(KH)]

        # hidden history, transposed: hist[k][128, BL*S], col = t*BL + b
        hist = []
        for k in range(KH):
            hist_t = hist_pool.tile([128, BL * S], mmdt, tag=f"hist{k}")
            hist.append(hist_t)

        # ---------- phase 1: xg = emb_aug @ wih (bias via ones col) ----
        with ExitStack() as ph1:
            wp = ph1.enter_context(tc.tile_pool(name="wih", bufs=1))
            wih_all = wp.tile([128, KD, G4], mmdt, tag="wih")
            nc.sync.dma_start(wih_all[:],
                              wih.rearrange("(k p) g -> p k g", p=128))
            wih_sb = [wih_all[:, k, :] for k in range(KD)]
            ep = ph1.enter_context(tc.tile_pool(name="emb", bufs=3))
            etp = ph1.enter_context(tc.tile_pool(name="embT", bufs=3))
            pp = ph1.enter_context(
                tc.tile_pool(name="ph1ps", bufs=2, space="PSUM"))
            xp = ph1.enter_context(
                tc.tile_pool(name="xgps", bufs=3, space="PSUM"))
            for rt in range(RT):
                emb_sb = ep.tile([128, DA], f32, tag="emb")
                nc.sync.dma_start(emb_sb[:], emb[rt * 128:(rt + 1) * 128, :])
                etps = pp.tile([128, 512], f32, tag="etps")
                etsb = etp.tile([128, DA], mmdt, tag="etsb")
                for k in range(KD):
                    ps = etps[:, (k % 4) * 128:(k % 4) * 128 + 128]
                    nc.tensor.transpose(ps, emb_sb[:, k * 128:(k + 1) * 128],
                                        id128_sb[:])
                    nc.vector.tensor_copy(etsb[:, k * 128:(k + 1) * 128], ps)
                b_idx, tq = rt // 4, rt % 4
                for n in range(3):
                    xps = xp.tile([128, 512], f32, tag="xps")
                    for k in range(KD):
                        nc.tensor.matmul(
                            xps[:],
                            etsb[:, k * 128:(k + 1) * 128],
                            wih_sb[k][:, n * 512:(n + 1) * 512],
                            start=(k == 0), stop=(k == KD - 1))
                    xsb = etp.tile([128, 512], f32, tag="xsb")
                    nc.vector.tensor_copy(xsb[:], xps[:])
                    # rows of this tile are t = tq*128 .. tq*128+127, one b
                    dst = xg_dram[tq * 128:(tq + 1) * 128, b_idx, n * 512:(n + 1) * 512]
                    nc.sync.dma_start(dst, xsb[:])

        # ---------- phase 2: LSTM scan ----------
        with ExitStack() as ph2:
            xgp = ph2.enter_context(tc.tile_pool(name="xgin", bufs=2))
            gp = ph2.enter_context(
                tc.tile_pool(name="gps", bufs=2, space="PSUM"))
            htp = ph2.enter_context(
                tc.tile_pool(name="htps", bufs=2, space="PSUM"))
            sp = ph2.enter_context(tc.tile_pool(name="scan", bufs=2))
            cp = ph2.enter_context(tc.tile_pool(name="cbuf", bufs=2))

            c_prev = c0_sb
            xg_sb = None
            for t in range(STEPS):
                tl = t % XG_CHUNK
                if tl == 0:
                    xg_sb = xgp.tile([BL, XG_CHUNK, G4], f32, tag="xg")
                    nc.sync.dma_start(
                        xg_sb[:],
                        xg_dram[t:t + XG_CHUNK, :, :].rearrange(
                            "t b g -> b t g"))
                if t == 0:
                    hT = [h0t_sb[k][:, :] for k in range(KH)]
                else:
                    hT = [hist[k][:, (t - 1) * BL:t * BL] for k in range(KH)]

                g_ps = gp.tile([BL, G4], f32, tag="g")
                for n in range(3):
                    for k in range(KH):
                        nc.tensor.matmul(
                            g_ps[:, n * 512:(n + 1) * 512],
                            hT[k],
                            whh_sb[k][:, n * 512:(n + 1) * 512],
                            start=(k == 0), stop=(k == KH - 1))
                g_sb = sp.tile([BL, G4], f32, tag="gsb")
                for n in range(3):
                    sl = slice(n * 512, (n + 1) * 512)
                    nc.vector.tensor_add(
                        g_sb[:, sl], g_ps[:, sl],
                        xg_sb[:, tl, n * 512:(n + 1) * 512])
                # gate order in weights was permuted to [i, f, o, g]
                a_sb = sp.tile([BL, G4], f32, tag="asb")
                nc.scalar.activation(a_sb[:, 0:3 * H], g_sb[:, 0:3 * H],
                                     AF.Sigmoid)
                nc.scalar.activation(a_sb[:, 3 * H:G4], g_sb[:, 3 * H:G4],
                                     AF.Tanh)
                c_new = cp.tile([BL, H], f32, tag="c")
                tmp = sp.tile([BL, 2 * H], f32, tag="tmp")
                nc.vector.tensor_mul(tmp[:, 0:H], a_sb[:, 0:H],
                                     a_sb[:, 3 * H:G4])          # i*tanh(g)
                nc.vector.tensor_mul(c_new[:], a_sb[:, H:2 * H], c_prev[:])
                nc.vector.tensor_add(c_new[:], c_new[:], tmp[:, 0:H])
                nc.scalar.activation(tmp[:, H:2 * H], c_new[:], AF.Tanh)
                h_sb = sp.tile([BL, H], f32, tag="h")
                nc.vector.tensor_mul(h_sb[:], a_sb[:, 2 * H:3 * H],
                                     tmp[:, H:2 * H])
                ht_ps = htp.tile([128, KH * BL], f32, tag="htps")
                for k in range(KH):
                    nc.tensor.transpose(ht_ps[:, k * BL:(k + 1) * BL],
                                        h_sb[:, k * 128:(k + 1) * 128],
                                        id16_sb[:])
                for k in range(KH):
                    nc.vector.tensor_copy(hist[k][:, t * BL:(t + 1) * BL],
                                          ht_ps[:, k * BL:(k + 1) * BL])
                c_prev = c_new

        # ---------- phase 3: feats partial = hist^T @ wlin ----------
        with ExitStack() as ph3:
            fp = ph3.enter_context(
                tc.tile_pool(name="fps", bufs=2, space="PSUM"))
            fsb = ph3.enter_context(tc.tile_pool(name="fsb", bufs=2))
            NCH = (BL * S) // 512  # 16 chunks of 512 (t,b) columns
            for j in range(NCH):
                f_ps = fp.tile([T, 512], f32, tag="f")
                for k in range(KH):
                    nc.tensor.matmul(
                        f_ps[:],
                        wlin_sb[k],
                        hist[k][:, j * 512:(j + 1) * 512],
                        start=(k == 0), stop=(k == KH - 1))
                f_sb = fsb.tile([T, 512], f32, tag="fsb")
                nc.vector.tensor_copy(f_sb[:], f_ps[:])
                nc.sync.dma_start(
                    partial[j * 512:(j + 1) * 512, :].rearrange("r t -> t r"),
                    f_sb[:])

    nc.compile()
    return nc


def _get_program():
    if "nc" not in _cache:
        _cache["nc"] = _build_program()
    return _cache["nc"]


# gate-order permutation: torch [i,f,g,o] -> kernel [i,f,o,g]
_PERM = np.concatenate([np.arange(0, H), np.arange(H, 2 * H),
                        np.arange(3 * H, 4 * H), np.arange(2 * H, 3 * H)])


def _prep_core_inputs(embeds, h0, c0, W_ih, W_hh, b_ih, b_hh, W_lin, reverse):
    """Build the per-core input map. embeds: [BL, S, D] already aligned."""
    e = embeds
    if reverse:
        e = e[:, ::-1, :]
    ea = np.zeros((BL, S, DA), np.float32)
    ea[:, :, :D] = e
    ea[:, :, D] = 1.0
    wih_a = np.zeros((DA, G4), np.float32)
    wih_a[:D, :] = W_ih.T[:, _PERM]
    wih_a[D, :] = (b_ih + b_hh)[_PERM]
    whh_t = np.ascontiguousarray(W_hh.T[:, _PERM], np.float32)
    half = slice(0, H) if not reverse else slice(H, 2 * H)
    wlin_t = np.ascontiguousarray(W_lin[:, half].T, np.float32)
    return {
        "emb": ea.reshape(BL * S, DA),
        "wih": wih_a,
        "whh": whh_t,
        "wlin": wlin_t,
        "h0t": np.ascontiguousarray(h0.T, np.float32),
        "c0": np.ascontiguousarray(c0, np.float32),
        "id16": np.eye(16, dtype=np.float32),
        "id128": np.eye(128, dtype=np.float32),
    }


def kernel(hidden_states, h0, c0, W_ih_f, W_hh_f, b_ih_f, b_hh_f,
           W_ih_b, W_hh_b, b_ih_b, b_hh_b, W_lin, b_lin, start_ids, masks,
           _trace=False):
    from concourse.bass_utils import run_bass_kernel_spmd

    hidden_states = np.asarray(hidden_states, np.float32)
    h0 = np.asarray(h0, np.float32)
    c0 = np.asarray(c0, np.float32)

    embeds = _align_np(hidden_states, start_ids, masks)

    in_maps = []
    for core in range(NC):
        rev = core >= 4
        q = core % 4
        bs = slice(q * BL, (q + 1) * BL)
        d = 1 if rev else 0
        W_ih = np.asarray(W_ih_b if rev else W_ih_f, np.float32)
        W_hh = np.asarray(W_hh_b if rev else W_hh_f, np.float32)
        b_i = np.asarray(b_ih_b if rev else b_ih_f, np.float32)
        b_h = np.asarray(b_hh_b if rev else b_hh_f, np.float32)
        in_maps.append(_prep_core_inputs(
            embeds[bs], h0[d, bs], c0[d, bs], W_ih, W_hh, b_i, b_h,
            np.asarray(W_lin, np.float32), rev))

    nc = _get_program()
    res = run_bass_kernel_spmd(nc, in_maps, list(range(NC)), trace=_trace)
    outs = res.results

    feats = np.zeros((B, S, T), np.float32)
    for q in range(4):
        bs = slice(q * BL, (q + 1) * BL)
        fwd = outs[q]["partial"].reshape(S, BL, T).transpose(1, 0, 2)
        bwd = outs[q + 4]["partial"].reshape(S, BL, T).transpose(1, 0, 2)
        feats[bs] = fwd + bwd[:, ::-1, :] + np.asarray(b_lin, np.float32)
    if _trace:
        return feats, res
    return feats

